# revision 4
# baseline (speedup 1.0000x reference)
"""AsyncLightBus Trainium2 kernel (8 NeuronCores, SPMD).

Computation (per batch b):
  summary = normalize((sum_s x[b]) @ Wp.T)              [128]
  aug     = [summary; bus_cache[b]]                     [25, 128]
  q       = x @ Wq.T / sqrt(128)                        [S, 128]
  attn    = softmax(q @ aug.T)                          [S, 25]
  out     = x + sigmoid(gate) * (attn @ aug) @ Wm.T     [S, 4096]
  new_cache = [bus_cache; summary]                      [25, 128]

Sharding: rows of x flattened to [B*S, D] = [16384, 4096], 2048 rows per
core (cores 0-3 -> batch 0, cores 4-7 -> batch 1). The only cross-core
dependency is the per-batch publish vector (128 floats/batch), combined
with a tiny AllReduce. Everything else is local.

Per-core dataflow:
  phase 1: stream x in (f32->bf16 cast during DMA), PE-transpose 128x128
           tiles to build x^T, accumulate q^T = Wq x^T and P^T = Wp x^T
           in PSUM; publish^T = rowsum(P^T).
  transition: AllReduce [2,128] publish partials, normalize -> summary.
  phase 2: scores^T = aug^T.T q^T, exp, denominators via ones-matmul,
           attn^T, gathered^T, modulation chunks = gathered^T.T Wm^T*sig,
           out = x_bf16 + modulation, DMA out (bf16->f32 cast).
"""

import os

import numpy as np

import concourse.bass as bass
import concourse.tile as tile
from concourse import bacc, mybir
from concourse.bass_utils import run_bass_kernel_spmd
from concourse.masks import make_identity

FP32 = mybir.dt.float32
BF16 = mybir.dt.bfloat16
AX_X = mybir.AxisListType.X
AF = mybir.ActivationFunctionType

B, S, D, K, L = 2, 8192, 4096, 128, 24
NCORES = 8
R = (B * S) // NCORES  # 2048 rows per core
SBLK = 512             # rows per phase block
NBLK = R // SBLK       # 4
NST = R // 128         # 16 s-tiles of 128 rows
NDT = D // 128         # 32 d-tiles
ND512 = D // 512       # 8
LA = L + 1             # 25 augmented slots
INV_SQRT_K = 1.0 / float(np.sqrt(K))

_CACHE = {}


def _build():
    nc = bacc.Bacc("TRN2", target_bir_lowering=False, debug=False,
                   num_devices=NCORES)

    x_d = nc.dram_tensor("x", [R, D], FP32, kind="ExternalInput").ap()
    bus_d = nc.dram_tensor("bus", [L, K], FP32, kind="ExternalInput").ap()
    wq_d = nc.dram_tensor("Wq", [K, D], FP32, kind="ExternalInput").ap()
    wp_d = nc.dram_tensor("Wp", [K, D], FP32, kind="ExternalInput").ap()
    wm_d = nc.dram_tensor("Wm", [D, K], FP32, kind="ExternalInput").ap()
    gate_d = nc.dram_tensor("gate", [1, 1], FP32, kind="ExternalInput").ap()
    eb_d = nc.dram_tensor("eb", [1, 2], FP32, kind="ExternalInput").ap()
    ebT_d = nc.dram_tensor("ebT", [2, 1], FP32, kind="ExternalInput").ap()

    out_d = nc.dram_tensor("out", [R, D], FP32, kind="ExternalOutput").ap()
    summ_d = nc.dram_tensor("summ", [B, K], FP32, kind="ExternalOutput").ap()

    with tile.TileContext(nc) as tc:
        with (
            tc.tile_pool(name="const", bufs=1) as const,
            tc.tile_pool(name="xt", bufs=4) as xtp,
            tc.tile_pool(name="outp", bufs=2) as outp,
            tc.tile_pool(name="small", bufs=2) as small,
            tc.tile_pool(name="psA", bufs=3, space="PSUM") as psA,
            tc.tile_pool(name="psB", bufs=2, space="PSUM") as psB,
            tc.tile_pool(name="psC", bufs=1, space="PSUM") as psC,
            tc.tile_pool(name="dram", bufs=1, space="DRAM") as dram,
        ):
            # ---------------- constants / small inputs ----------------
            ident_bf = const.tile([128, 128], BF16, tag="identb", name="ident_bf")
            make_identity(nc, ident_bf)
            ident_f = const.tile([128, 128], FP32, tag="identf", name="ident_f")
            make_identity(nc, ident_f)

            ones_la = const.tile([LA, 1], BF16, tag="ones_la", name="ones_la")
            nc.gpsimd.memset(ones_la, 1.0)
            ones_1la = const.tile([1, LA], FP32, tag="ones_1la", name="ones_1la")
            nc.gpsimd.memset(ones_1la, 1.0)
            ones_1_128 = const.tile([1, 128], FP32, tag="ones128", name="ones_1_128")
            nc.gpsimd.memset(ones_1_128, 1.0)

            eb_sb = const.tile([1, 2], FP32, tag="eb", name="eb_sb")
            nc.sync.dma_start(out=eb_sb, in_=eb_d)
            ebT_sb = const.tile([2, 1], FP32, tag="ebT", name="ebT_sb")
            nc.sync.dma_start(out=ebT_sb, in_=ebT_d)

            gate_sb = const.tile([1, 1], FP32, tag="gate", name="gate_sb")
            nc.sync.dma_start(out=gate_sb, in_=gate_d)
            sg = const.tile([1, 1], FP32, tag="sg", name="sg")
            nc.scalar.activation(sg, gate_sb, AF.Sigmoid)
            # replicate sigmoid(gate) across 128 partitions via ones matmul
            sgr_ps = psA.tile([128, 1], FP32, tag="tr", name="sgr_ps")
            nc.tensor.matmul(sgr_ps, ones_1_128, sg, start=True, stop=True)
            sg_rep = const.tile([128, 1], FP32, tag="sgrep", name="sg_rep")
            nc.scalar.activation(sg_rep, sgr_ps, AF.Copy)

            # ---------------- weights: transposed copies ----------------
            # WqT / WpT: [d=128, k=128] tiles packed as [128, 32*128]
            wqT = const.tile([128, D], BF16, tag="wqT", name="wqT")
            wpT = const.tile([128, D], BF16, tag="wpT", name="wpT")
            wmT = const.tile([128, D], BF16, tag="wmT", name="wmT")

            for (w_d, w_t) in ((wq_d, wqT), (wp_d, wpT)):
                w_nat = outp.tile([128, D], BF16, tag="outsb",
                                  name=f"wnat_{w_t.tensor.name}")
                nc.gpsimd.dma_start(out=w_nat, in_=w_d)  # f32 -> bf16 cast
                for dt in range(NDT):
                    pt = psA.tile([128, 128], BF16, tag="tr", name="w_tr")
                    nc.tensor.transpose(
                        pt, w_nat[:, dt * 128:(dt + 1) * 128], ident_bf)
                    nc.scalar.activation(
                        w_t[:, dt * 128:(dt + 1) * 128], pt, AF.Copy)

            # Wm [4096, 128] -> wm_nat [128, (t k)] then transpose each tile
            wm_nat = outp.tile([128, D], BF16, tag="outsb", name="wm_nat")
            nc.gpsimd.dma_start(
                out=wm_nat.rearrange("p (t k) -> p t k", k=K),
                in_=wm_d.rearrange("(t p) k -> p t k", p=128))
            wmT_tmp = outp.tile([128, D], BF16, tag="outsb", name="wmT_tmp")
            for dt in range(NDT):
                pt = psA.tile([128, 128], BF16, tag="tr", name="wm_tr")
                nc.tensor.transpose(
                    pt, wm_nat[:, dt * 128:(dt + 1) * 128], ident_bf)
                nc.scalar.activation(
                    wmT_tmp[:, dt * 128:(dt + 1) * 128], pt, AF.Copy)
            # fold sigmoid(gate) into WmT
            nc.vector.tensor_scalar_mul(wmT, wmT_tmp, sg_rep)

            # ---------------- aug (bus part) ----------------
            aug_nat = const.tile([LA, 128], BF16, tag="aug", name="aug_nat")
            nc.gpsimd.dma_start(out=aug_nat[1:LA, :], in_=bus_d)  # cast
            bus_sb = const.tile([L, 128], BF16, tag="bus", name="bus_sb")
            nc.gpsimd.dma_start(out=bus_sb, in_=bus_d)  # cast, partition 0
            augT = const.tile([128, LA], BF16, tag="augT", name="augT")
            busT_ps = psA.tile([128, 128], BF16, tag="tr", name="busT_ps")
            nc.tensor.transpose(
                busT_ps[:, 0:L], bus_sb, ident_bf[0:L, 0:L])
            nc.scalar.activation(augT[:, 1:LA], busT_ps[:, 0:L], AF.Copy)

            # ---------------- phase 1: stream x ----------------
            stash = const.tile([128, NST * D], BF16, tag="stash", name="stash")
            # 4MB DMAs covering 256 rows (2 s-tiles) each
            for h in range(NST // 2):
                nc.gpsimd.dma_start(
                    out=stash[:, h * 2 * D:(h + 1) * 2 * D].rearrange(
                        "p (a d) -> p a d", a=2),
                    in_=x_d[h * 256:(h + 1) * 256, :].rearrange(
                        "(a p) d -> p a d", p=128))

            q_all = const.tile([128, R], BF16, tag="qall", name="q_all")
            pubacc = const.tile([128, NBLK], FP32, tag="pubacc", name="pubacc")

            for sb in range(NBLK):
                q_ps = psB.tile([128, SBLK], FP32, tag="acc", name="q_ps")
                p_ps = psC.tile([128, SBLK], FP32, tag="pacc", name="p_ps")
                for dt in range(NDT):
                    xt_buf = xtp.tile([128, SBLK], BF16, tag="xt", name="xt_buf")
                    for si in range(SBLK // 128):
                        st = sb * (SBLK // 128) + si
                        off = st * D + dt * 128
                        ptr = psA.tile([128, 128], BF16, tag="tr", name="x_tr")
                        nc.tensor.transpose(
                            ptr, stash[:, off:off + 128], ident_bf)
                        nc.scalar.activation(
                            xt_buf[:, si * 128:(si + 1) * 128], ptr, AF.Copy)
                    nc.tensor.matmul(
                        q_ps, wqT[:, dt * 128:(dt + 1) * 128], xt_buf,
                        start=(dt == 0), stop=(dt == NDT - 1))
                    nc.tensor.matmul(
                        p_ps, wpT[:, dt * 128:(dt + 1) * 128], xt_buf,
                        start=(dt == 0), stop=(dt == NDT - 1))
                nc.scalar.activation(
                    q_all[:, sb * SBLK:(sb + 1) * SBLK], q_ps, AF.Copy,
                    scale=INV_SQRT_K)
                nc.vector.reduce_sum(
                    out=pubacc[:, sb:sb + 1], in_=p_ps, axis=AX_X)

            # ---------------- transition: publish + AllReduce ----------------
            padd1 = small.tile([128, 1], FP32, tag="pubt", name="padd1")
            nc.vector.tensor_add(padd1, pubacc[:, 0:1], pubacc[:, 1:2])
            padd2 = small.tile([128, 1], FP32, tag="pubt", name="padd2")
            nc.vector.tensor_add(padd2, pubacc[:, 2:3], pubacc[:, 3:4])
            pubT = small.tile([128, 1], FP32, tag="pubt2", name="pubT")
            nc.vector.tensor_add(pubT, padd1, padd2)

            prow_ps = psA.tile([1, 128], FP32, tag="tr", name="prow_ps")
            nc.tensor.transpose(prow_ps, pubT, ident_f)
            pubrow = small.tile([1, 128], FP32, tag="prow", name="pubrow")
            nc.scalar.activation(pubrow, prow_ps, AF.Copy)

            pub2_ps = psA.tile([2, 128], FP32, tag="tr", name="pub2_ps")
            nc.tensor.matmul(pub2_ps, eb_sb, pubrow, start=True, stop=True)
            pub2 = small.tile([2, 128], FP32, tag="p2", name="pub2")
            nc.scalar.activation(pub2, pub2_ps, AF.Copy)

            cc_in = dram.tile([2, 128], FP32, name="cc_in")
            cc_out = dram.tile([2, 128], FP32, addr_space="Shared", name="cc_out")
            nc.sync.dma_start(out=cc_in, in_=pub2)
            nc.gpsimd.collective_compute(
                "AllReduce", mybir.AluOpType.add,
                replica_groups=[list(range(NCORES))],
                ins=[cc_in.opt()], outs=[cc_out.opt()])
            pub2r = small.tile([2, 128], FP32, tag="p2r", name="pub2r")
            nc.sync.dma_start(out=pub2r, in_=cc_out)

            sq = small.tile([2, 128], FP32, tag="p2s", name="sq")
            nc.vector.tensor_mul(sq, pub2r, pub2r)
            nrm2 = small.tile([2, 1], FP32, tag="n2", name="nrm2")
            nc.vector.reduce_sum(out=nrm2, in_=sq, axis=AX_X)
            nrm = small.tile([2, 1], FP32, tag="n", name="nrm")
            nc.scalar.activation(nrm, nrm2, AF.Sqrt)
            rinv = small.tile([2, 1], FP32, tag="ri", name="rinv")
            nc.vector.reciprocal(rinv, nrm)
            summ2 = small.tile([2, 128], FP32, tag="s2", name="summ2")
            nc.vector.tensor_scalar_mul(summ2, pub2r, rinv)
            nc.sync.dma_start(out=summ_d, in_=summ2)

            # own-batch summary into aug (both layouts)
            sown_ps = psA.tile([1, 128], FP32, tag="tr", name="sown_ps")
            nc.tensor.matmul(sown_ps, ebT_sb, summ2, start=True, stop=True)
            nc.scalar.activation(aug_nat[0:1, :], sown_ps, AF.Copy)
            sownT_ps = psA.tile([128, 1], FP32, tag="tr", name="sownT_ps")
            nc.tensor.matmul(sownT_ps, summ2, ebT_sb, start=True, stop=True)
            nc.scalar.activation(augT[:, 0:1], sownT_ps, AF.Copy)

            # ---------------- phase 2: attention + output ----------------
            for sb in range(NBLK):
                qs = q_all[:, sb * SBLK:(sb + 1) * SBLK]
                sc_ps = psA.tile([LA, SBLK], FP32, tag="tr", name="sc_ps")
                nc.tensor.matmul(sc_ps, augT, qs, start=True, stop=True)
                e = small.tile([LA, SBLK], BF16, tag="e", name="e")
                nc.scalar.activation(e, sc_ps, AF.Exp)

                dn_ps = psC.tile([1, SBLK], FP32, tag="pacc", name="dn_ps")
                nc.tensor.matmul(dn_ps, ones_la, e, start=True, stop=True)
                dnr = small.tile([1, SBLK], FP32, tag="dnr", name="dnr")
                nc.vector.reciprocal(dnr, dn_ps)
                dnrep_ps = psA.tile([LA, SBLK], FP32, tag="tr", name="dnrep_ps")
                nc.tensor.matmul(dnrep_ps, ones_1la, dnr, start=True, stop=True)
                en = small.tile([LA, SBLK], BF16, tag="en", name="en")
                nc.vector.tensor_mul(en, e, dnrep_ps)

                gt_ps = psB.tile([128, SBLK], FP32, tag="acc", name="gt_ps")
                nc.tensor.matmul(gt_ps, aug_nat, en, start=True, stop=True)
                gt = small.tile([128, SBLK], BF16, tag="gt", name="gt")
                nc.scalar.activation(gt, gt_ps, AF.Copy)

                for ch in range(SBLK // 128):
                    st = sb * (SBLK // 128) + ch
                    out_sb = outp.tile([128, D], BF16, tag="outsb", name="out_sb")
                    for dsl in range(ND512):
                        mod_ps = psB.tile([128, 512], FP32, tag="mod",
                                          name="mod_ps")
                        nc.tensor.matmul(
                            mod_ps, gt[:, ch * 128:(ch + 1) * 128],
                            wmT[:, dsl * 512:(dsl + 1) * 512],
                            start=True, stop=True)
                        off = st * D + dsl * 512
                        nc.vector.tensor_add(
                            out_sb[:, dsl * 512:(dsl + 1) * 512],
                            mod_ps, stash[:, off:off + 512])
                    nc.gpsimd.dma_start(
                        out=out_d[st * 128:(st + 1) * 128, :], in_=out_sb)

    nc.compile()
    return nc


def _get_nc():
    if "nc" not in _CACHE:
        _CACHE["nc"] = _build()
    return _CACHE["nc"]


def _make_in_maps(x, bus_cache, Wp, Wq, Wm, gate):
    xf = np.ascontiguousarray(np.asarray(x, dtype=np.float32).reshape(B * S, D))
    bus = np.asarray(bus_cache, dtype=np.float32)
    Wp = np.ascontiguousarray(np.asarray(Wp, dtype=np.float32))
    Wq = np.ascontiguousarray(np.asarray(Wq, dtype=np.float32))
    Wm = np.ascontiguousarray(np.asarray(Wm, dtype=np.float32))
    g = np.asarray(gate, dtype=np.float32).reshape(1, 1)

    in_maps = []
    for c in range(NCORES):
        b = c // (NCORES // B)
        eb = np.zeros((1, 2), np.float32)
        eb[0, b] = 1.0
        in_maps.append({
            "x": np.ascontiguousarray(xf[c * R:(c + 1) * R]),
            "bus": np.ascontiguousarray(bus[b]),
            "Wq": Wq, "Wp": Wp, "Wm": Wm, "gate": g,
            "eb": eb, "ebT": np.ascontiguousarray(eb.T),
        })
    return in_maps


def run(trace=False, **inputs):
    """Returns (out, new_cache), BassKernelResults."""
    nc = _get_nc()
    in_maps = _make_in_maps(**inputs)
    res = run_bass_kernel_spmd(
        nc, in_maps, core_ids=list(range(NCORES)), trace=trace)
    out = np.concatenate(
        [res.results[c]["out"] for c in range(NCORES)], axis=0)
    out = out.reshape(B, S, D)
    summary = res.results[0]["summ"]  # [B, K]
    bus = np.asarray(inputs["bus_cache"], dtype=np.float32)
    new_cache = np.concatenate([bus, summary[:, None, :]], axis=1)
    return (out, new_cache), res


def kernel(x, bus_cache, Wp, Wq, Wm, gate):
    (out, new_cache), _ = run(
        trace=bool(os.environ.get("KERNEL_TRACE")),
        x=x, bus_cache=bus_cache, Wp=Wp, Wq=Wq, Wm=Wm, gate=gate)
    return out, new_cache
